# revision 5
# baseline (speedup 1.0000x reference)
"""MLA (multi-head latent attention) Trainium2 Bass kernel, v2.

Sharding: 8 cores = batch(2) x head-groups(4 heads each).
Latent projections computed on seq-quarters within a batch group and
all-gathered; heads tensor-parallel; final projection row-split with
host-side partial sum.

v2 vs v1:
- Attention PV uses V as the stationary operand with PSUM accumulation
  over j-tiles -> 96 wide matmuls instead of 544 narrow ones, and the
  output lands pre-transposed [d, q] so the 64 PE transposes are gone.
- Softmax denominator accumulated on DVE (f32) + one ones-matmul.
- 1024-wide moving operands for scores/PV/out-projection/kr halve the
  PE instruction count (sequencer was the bottleneck).
- Single [128,128] triangle applies the causal mask at every diagonal
  tile; sub-diagonal columns are never computed.
- Phase-B weights DMA'd at program start on the scalar queue so the
  collective stall doesn't serialize them.
"""

import sys
import numpy as np
import ml_dtypes

for _p in ("/opt/trn_rl_repo", "/root/.axon_site/_ro/trn_rl_repo"):
    if _p not in sys.path:
        sys.path.append(_p)

BF16 = ml_dtypes.bfloat16

D_MODEL = 2048
SEQ = 2048
BATCH = 2
N_HEADS = 16
D_HEAD = 128
D_KV = 512
D_ROPE = 64
ROPE_BASE = 10000.0
EPS = 1e-5
H_LOC = 4          # heads per core
N_CORES = 8

_BUILD_CACHE = {}


def build_program(reps: int = 1, split_coll: bool = True):
    key = (reps, split_coll)
    if key in _BUILD_CACHE:
        return _BUILD_CACHE[key]

    import concourse.bass as bass  # noqa: F401
    import concourse.mybir as mybir
    from concourse import bacc
    from concourse.tile import TileContext
    from contextlib import ExitStack

    f32 = mybir.dt.float32
    bf16 = mybir.dt.bfloat16
    AF = mybir.ActivationFunctionType
    OP = mybir.AluOpType

    nc = bacc.Bacc(num_devices=8)

    xT = nc.declare_dram_parameter("xT", [D_MODEL, SEQ], bf16, isOutput=False)
    wdq = nc.declare_dram_parameter("wdq", [D_MODEL, D_KV], bf16, isOutput=False)
    wdkv = nc.declare_dram_parameter("wdkv", [D_MODEL, D_KV], bf16, isOutput=False)
    wq = nc.declare_dram_parameter("wq", [D_KV, H_LOC * 128], bf16, isOutput=False)
    wuk2 = nc.declare_dram_parameter("wuk2", [D_KV, 2 * 128], bf16, isOutput=False)
    wkr2 = nc.declare_dram_parameter("wkr2", [D_MODEL, 2 * 128], bf16, isOutput=False)
    wuv = nc.declare_dram_parameter("wuv", [D_KV, H_LOC * 128], bf16, isOutput=False)
    wout = nc.declare_dram_parameter("wout", [H_LOC * 128, D_MODEL], bf16, isOutput=False)
    xq = nc.declare_dram_parameter("xq", [D_MODEL, 512], bf16, isOutput=False)
    mult = nc.declare_dram_parameter("mult", [128, 2 * SEQ], bf16, isOutput=False)
    tri = nc.declare_dram_parameter("tri", [128, 128], bf16, isOutput=False)
    y = nc.declare_dram_parameter("y", [SEQ, D_MODEL], bf16, isOutput=True)
    if split_coll:
        csrc_kv = nc.dram_tensor("csrc_kv", [D_KV, 512], bf16)
        cdst_kv = nc.dram_tensor("cdst_kv", [4, D_KV, 512], bf16)
        csrc_q = nc.dram_tensor("csrc_q", [D_KV, 512], bf16)
        cdst_q = nc.dram_tensor("cdst_q", [4, D_KV, 512], bf16)
    else:
        csrc = nc.dram_tensor("csrc", [2 * D_KV, 512], bf16)
        cdst = nc.dram_tensor("cdst", [4, 2 * D_KV, 512], bf16)

    SCALE = 1.0 / float(np.sqrt(np.float32(D_HEAD)))
    NKT = D_MODEL // 128    # 16 k-tiles over d_model
    NLT = D_KV // 128       # 4  k-tiles over latent
    NSN = SEQ // 512        # 4  sequence chunks (gather granularity)
    NB = SEQ // 1024        # 2  attention q-blocks

    with TileContext(nc) as tc, ExitStack() as top:
        pp = top.enter_context(tc.tile_pool(name="persist", bufs=1))
        qt_sb = pp.tile([128, H_LOC * SEQ], bf16, tag="qt")
        kt_sb = pp.tile([128, H_LOC * SEQ], bf16, tag="kt")
        v_sb = pp.tile([128, (SEQ // 128) * H_LOC * 128], bf16, tag="v")
        tri_sb = pp.tile([128, 128], bf16, tag="tri")
        ones_sb = pp.tile([128, 1], bf16, tag="ones")

        for _rep in range(reps):
            # early loads on the scalar queue (off the critical path)
            nc.scalar.dma_start(out=tri_sb[:], in_=tri[:, :])
            nc.gpsimd.memset(ones_sb[:], 1.0)

            # ---------------- Phase A: latents + projections ----------------
            with ExitStack() as pa:
                pA = pa.enter_context(tc.tile_pool(name="phA", bufs=1))
                psX = pa.enter_context(tc.tile_pool(name="psX", bufs=2, space="PSUM"))
                pakr = pa.enter_context(ExitStack())
                pAk = pakr.enter_context(tc.tile_pool(name="phAk", bufs=1))
                pXs = pakr.enter_context(tc.tile_pool(name="pXs", bufs=2))
                pa1 = pa.enter_context(ExitStack())
                pA1 = pa1.enter_context(tc.tile_pool(name="phA1", bufs=1))
                pCp = pa1.enter_context(tc.tile_pool(name="pCp", bufs=4))
                pCn = pa1.enter_context(tc.tile_pool(name="pCn", bufs=2))
                pSq = pa1.enter_context(tc.tile_pool(name="pSq", bufs=1))
                pSt = pa1.enter_context(tc.tile_pool(name="pSt", bufs=1))
                psS = pa1.enter_context(tc.tile_pool(name="psS", bufs=2, space="PSUM"))

                xq_sb = pA1.tile([128, NKT * 512], bf16, tag="xq")
                wdq_sb = pA1.tile([128, NKT * D_KV], bf16, tag="wdq")
                wdkv_sb = pA1.tile([128, NKT * D_KV], bf16, tag="wdkv")
                eps_sb = pA1.tile([1, 1], f32, tag="eps")
                cq_full = pA.tile([128, NLT * SEQ], bf16, tag="cqf")
                ckv_full = pA.tile([128, NLT * SEQ], bf16, tag="ckvf")
                wq_sb = pA.tile([128, NLT * 512], bf16, tag="wq")
                wuk2_sb = pA.tile([128, NLT * 256], bf16, tag="wuk2")
                wkr2_sb = pAk.tile([128, NKT * 256], bf16, tag="wkr2")
                wuv_sb = pA.tile([128, NLT * 512], bf16, tag="wuv")
                mult_sb = pA.tile([128, 2 * SEQ], bf16, tag="mult")

                def merged(dram, cols):
                    return dram.rearrange("(kt p) s -> p kt s", p=128)

                for kq in range(4):
                    nc.sync.dma_start(
                        out=xq_sb.rearrange("p (kt s) -> p kt s", s=512)[:, kq * 4:(kq + 1) * 4],
                        in_=xq[kq * 512:(kq + 1) * 512, :].rearrange(
                            "(kt p) s -> p kt s", p=128))
                for lt in range(NLT):
                    nc.scalar.dma_start(
                        out=wdkv_sb.rearrange("p (kt s) -> p kt s", s=D_KV)[:, :, lt * 128:(lt + 1) * 128],
                        in_=wdkv[:, lt * 128:(lt + 1) * 128].rearrange(
                            "(kt p) s -> p kt s", p=128))
                for lt in range(NLT):
                    nc.scalar.dma_start(
                        out=wdq_sb.rearrange("p (kt s) -> p kt s", s=D_KV)[:, :, lt * 128:(lt + 1) * 128],
                        in_=wdq[:, lt * 128:(lt + 1) * 128].rearrange(
                            "(kt p) s -> p kt s", p=128))
                nc.scalar.dma_start(
                    out=wkr2_sb.rearrange("p (kt s) -> p kt s", s=256),
                    in_=merged(wkr2, 256))
                nc.scalar.dma_start(
                    out=wq_sb.rearrange("p (lt s) -> p lt s", s=512),
                    in_=wq.rearrange("(lt p) s -> p lt s", p=128))
                nc.scalar.dma_start(
                    out=wuk2_sb.rearrange("p (lt s) -> p lt s", s=256),
                    in_=wuk2.rearrange("(lt p) s -> p lt s", p=128))
                nc.scalar.dma_start(
                    out=wuv_sb.rearrange("p (lt s) -> p lt s", s=512),
                    in_=wuv.rearrange("(lt p) s -> p lt s", p=128))
                nc.scalar.dma_start(out=mult_sb[:], in_=mult[:, :])
                nc.gpsimd.memset(eps_sb[:], EPS)

                # --- local latent quarter from xq, rmsnorm, kv first ---
                for ci, (cname, wd_sb) in enumerate((("kv", wdkv_sb), ("q", wdq_sb))):
                    cps_l = []
                    sq = pSq.tile([128, NLT * 512], bf16, tag="sq", name=f"sq{cname}")
                    for lt in range(NLT):
                        cp = psX.tile([128, 512], f32, tag="mm")
                        for kt in range(NKT):
                            nc.tensor.matmul(
                                cp[:],
                                wd_sb[:, kt * D_KV + lt * 128: kt * D_KV + (lt + 1) * 128],
                                xq_sb[:, kt * 512:(kt + 1) * 512],
                                start=(kt == 0), stop=(kt == NKT - 1))
                        cps = pCp.tile([128, 512], f32, tag="cpre", name=f"cpre{cname}{lt}")
                        nc.vector.tensor_copy(cps[:], cp[:])
                        nc.vector.tensor_tensor(sq[:, lt * 512:(lt + 1) * 512],
                                                cps[:], cps[:], OP.mult)
                        cps_l.append(cps)
                    ss = psS.tile([1, 512], f32, tag="stat")
                    for lt in range(NLT):
                        nc.tensor.matmul(ss[:], ones_sb[:],
                                         sq[:, lt * 512:(lt + 1) * 512],
                                         start=(lt == 0), stop=(lt == NLT - 1))
                    lnv = pSt.tile([1, 512], f32, tag="ln")
                    nc.scalar.activation(lnv[:], ss[:], AF.Ln, scale=1.0 / D_KV, bias=eps_sb[:])
                    rstd = pSt.tile([1, 512], f32, tag="rstd")
                    nc.scalar.activation(rstd[:], lnv[:], AF.Exp, scale=-0.5)
                    bstd = pSt.tile([128, 512], f32, tag="bstd")
                    nc.gpsimd.partition_broadcast(bstd[:], rstd[:])
                    cn = pCn.tile([128, NLT * 512], bf16, tag="cnloc", name=f"cnloc{cname}")
                    for lt in range(NLT):
                        nc.vector.tensor_tensor(cn[:, lt * 512:(lt + 1) * 512],
                                                cps_l[lt][:], bstd[:], OP.mult)
                    cn_v = cn.rearrange("p (lt s) -> p lt s", s=512)
                    if split_coll:
                        cs = csrc_kv if cname == "kv" else csrc_q
                        nc.sync.dma_start(
                            out=cs.rearrange("(lt p) s -> p lt s", p=128), in_=cn_v)
                    else:
                        nc.sync.dma_start(
                            out=csrc[ci * D_KV:(ci + 1) * D_KV, :].rearrange(
                                "(lt p) s -> p lt s", p=128), in_=cn_v)
                    if split_coll and cname == "kv":
                        nc.gpsimd.collective_compute(
                            "AllGather", OP.bypass,
                            replica_groups=[[0, 1, 2, 3], [4, 5, 6, 7]],
                            ins=[csrc_kv[:, :]], outs=[cdst_kv[:, :, :]])

                if split_coll:
                    nc.gpsimd.collective_compute(
                        "AllGather", OP.bypass,
                        replica_groups=[[0, 1, 2, 3], [4, 5, 6, 7]],
                        ins=[csrc_q[:, :]], outs=[cdst_q[:, :, :]])
                else:
                    nc.gpsimd.collective_compute(
                        "AllGather", OP.bypass,
                        replica_groups=[[0, 1, 2, 3], [4, 5, 6, 7]],
                        ins=[csrc[:, :]], outs=[cdst[:, :, :]])

                pa1.close()

                # --- kr projections from streamed xT (fills collective window) ---
                for half in range(2):
                    kps = [psX.tile([128, 1024], f32, tag="mm", name=f"kr{half}{p}")
                           for p in range(2)]
                    for kq in range(4):
                        xtile = pXs.tile([128, 4 * 1024], bf16, tag="xs", name=f"xs{half}{kq}")
                        nc.sync.dma_start(
                            out=xtile.rearrange("p (kt s) -> p kt s", s=1024),
                            in_=xT[kq * 512:(kq + 1) * 512,
                                   half * 1024:(half + 1) * 1024].rearrange(
                                       "(kt p) s -> p kt s", p=128))
                        for ki in range(4):
                            kt = kq * 4 + ki
                            for p in range(2):
                                for hf in range(2):
                                    nc.tensor.matmul(
                                        kps[p][:, hf * 512:(hf + 1) * 512],
                                        wkr2_sb[:, kt * 256 + p * 128: kt * 256 + (p + 1) * 128],
                                        xtile[:, ki * 1024 + hf * 512: ki * 1024 + (hf + 1) * 512],
                                        start=(kt == 0), stop=(kt == NKT - 1))
                    for p in range(2):
                        h0, h1 = 2 * p, 2 * p + 1
                        kp = kps[p]
                        a, z = half * 1024, (half + 1) * 1024
                        m0 = mult_sb[:, 0 * SEQ + a: 0 * SEQ + z]
                        m1 = mult_sb[:, 1 * SEQ + a: 1 * SEQ + z]
                        k0 = kt_sb[:, h0 * SEQ + a: h0 * SEQ + z]
                        k1 = kt_sb[:, h1 * SEQ + a: h1 * SEQ + z]
                        nc.vector.tensor_tensor(k0[64:128, :], kp[64:128, :], m0[64:128, :], OP.mult)
                        nc.vector.tensor_tensor(k1[0:64, :], kp[0:64, :], m1[0:64, :], OP.mult)

                pakr.close()
                psY = pa.enter_context(tc.tile_pool(name="psY", bufs=2, space="PSUM"))

                # --- unpack gathered latents (kv first, sn-major) ---
                for ci, cfull in ((0, ckv_full), (1, cq_full)):
                    cf_v = cfull.rearrange("p (lt sn s) -> p lt sn s", sn=NSN, s=512)
                    for sn in range(NSN):
                        if split_coll:
                            cd = cdst_kv if ci == 0 else cdst_q
                            in_v = cd[sn].rearrange("(lt p) s -> p lt s", p=128)
                        else:
                            in_v = cdst[sn, ci * D_KV:(ci + 1) * D_KV, :].rearrange(
                                "(lt p) s -> p lt s", p=128)
                        nc.sync.dma_start(out=cf_v[:, :, sn], in_=in_v)

                # --- K/V/Q projections, emitted per seq-half so block-0
                # attention can start while the second half projects ---
                def emit_kchain(pr, p):
                    a, z = pr * 1024, (pr + 1) * 1024
                    if True:
                        h0, h1 = 2 * p, 2 * p + 1
                        up = psX.tile([128, 1024], f32, tag="mm", name=f"uk{pr}{p}")
                        for lt in range(NLT):
                            for hf in range(2):
                                nc.tensor.matmul(
                                    up[:, hf * 512:(hf + 1) * 512],
                                    wuk2_sb[:, lt * 256 + p * 128: lt * 256 + (p + 1) * 128],
                                    ckv_full[:, lt * SEQ + a + hf * 512:
                                             lt * SEQ + a + (hf + 1) * 512],
                                    start=(lt == 0), stop=(lt == NLT - 1))
                        m0 = mult_sb[:, 0 * SEQ + a: 0 * SEQ + z]
                        m1 = mult_sb[:, 1 * SEQ + a: 1 * SEQ + z]
                        k0 = kt_sb[:, h0 * SEQ + a: h0 * SEQ + z]
                        k1 = kt_sb[:, h1 * SEQ + a: h1 * SEQ + z]
                        nc.vector.tensor_tensor(k0[0:64, :], up[0:64, :], m0[0:64, :], OP.mult)
                        nc.vector.tensor_tensor(k1[64:128, :], up[64:128, :], m1[64:128, :], OP.mult)

                def emit_vproj(sn):
                    for st in range(4):
                        s_tile = sn * 4 + st
                        vp = psY.tile([128, 512], f32, tag="vo")
                        for lt in range(NLT):
                            nc.tensor.matmul(
                                vp[:],
                                ckv_full[:, lt * SEQ + sn * 512 + st * 128:
                                         lt * SEQ + sn * 512 + (st + 1) * 128],
                                wuv_sb[:, lt * 512:(lt + 1) * 512],
                                start=(lt == 0), stop=(lt == NLT - 1))
                        nc.vector.tensor_copy(
                            v_sb[:, s_tile * 512:(s_tile + 1) * 512], vp[:])

                def emit_qchain(pr, hl):
                    a, z = pr * 1024, (pr + 1) * 1024
                    if True:
                        qp = psX.tile([128, 1024], f32, tag="mm", name=f"q{pr}{hl}")
                        for lt in range(NLT):
                            for hf in range(2):
                                nc.tensor.matmul(
                                    qp[:, hf * 512:(hf + 1) * 512],
                                    wq_sb[:, lt * 512 + hl * 128: lt * 512 + (hl + 1) * 128],
                                    cq_full[:, lt * SEQ + a + hf * 512:
                                            lt * SEQ + a + (hf + 1) * 512],
                                    start=(lt == 0), stop=(lt == NLT - 1))
                        nc.vector.tensor_tensor(
                            qt_sb[:, hl * SEQ + a: hl * SEQ + z],
                            qp[:], mult_sb[:, (hl % 2) * SEQ + a: (hl % 2) * SEQ + z],
                            OP.mult)

                emit_kchain(0, 0)
                emit_kchain(0, 1)
                for sn in range(NSN):
                    emit_vproj(sn)
                for hl in range(H_LOC):
                    emit_qchain(0, hl)
                # second-half K/Q chains are woven between block-0 attention
                # heads below
                fillers = [lambda p=p: emit_kchain(1, p) for p in range(2)]
                fillers += [lambda hl=hl: emit_qchain(1, hl) for hl in range(H_LOC)]

            # ------------- Phase B: attention + output projection -------------
            # (same pool scope as phase A so attention interleaves with the
            # trailing projections; psX/psY carry the PSUM traffic)
            if True:
                pb = pa.enter_context(ExitStack())
                pB = pb.enter_context(tc.tile_pool(name="pB", bufs=1))
                pEs = pb.enter_context(tc.tile_pool(name="pEs", bufs=6))
                pZa = pb.enter_context(tc.tile_pool(name="pZa", bufs=2))
                pZb = pb.enter_context(tc.tile_pool(name="pZb", bufs=2))
                pOt = pb.enter_context(tc.tile_pool(name="pOt", bufs=2))
                pYs = pb.enter_context(tc.tile_pool(name="pYs", bufs=2))
                pSm = pb.enter_context(tc.tile_pool(name="pSm", bufs=2))

                wout_sb = pB.tile([128, NLT * D_MODEL], bf16, tag="wout")
                nc.scalar.dma_start(
                    out=wout_sb.rearrange("p (f s) -> p f s", s=D_MODEL),
                    in_=wout.rearrange("(f p) s -> p f s", p=128))

                for b in range(NB):
                    njt = (b + 1) * 8
                    otc = [pOt.tile([128, 1024], bf16, tag=f"otc{f}", name=f"otc{b}_{f}")
                           for f in range(H_LOC)]
                    for hl in range(H_LOC):
                        oT = psY.tile([128, 1024], f32, tag="vo", name=f"oT{b}{hl}")
                        zacc = [pZa.tile([128, 1024], bf16, tag=f"zacc{par}",
                                         name=f"za{b}{hl}{par}") for par in range(2)]
                        sps, ess = {}, {}

                        def emit_score(jt):
                            kd = jt - b * 8
                            c0 = max(kd, 0) * 128
                            sp = psX.tile([128, 1024], f32, tag="mm", name=f"s{b}{hl}{jt}")
                            ksl = kt_sb[:, hl * SEQ + jt * 128: hl * SEQ + (jt + 1) * 128]
                            if c0 < 512:
                                nc.tensor.matmul(
                                    sp[:, c0:512],
                                    ksl,
                                    qt_sb[:, hl * SEQ + b * 1024 + c0:
                                          hl * SEQ + b * 1024 + 512],
                                    start=True, stop=True)
                            c0r = max(c0, 512)
                            nc.tensor.matmul(
                                sp[:, c0r:],
                                ksl,
                                qt_sb[:, hl * SEQ + b * 1024 + c0r: hl * SEQ + (b + 1) * 1024],
                                start=True, stop=True)
                            sps[jt] = sp

                        def emit_exp_pv(jt):
                            kd = jt - b * 8
                            c0 = max(kd, 0) * 128
                            sp = sps.pop(jt)
                            es = pEs.tile([128, 1024], bf16, tag="es", name=f"e{b}{hl}{jt}")
                            nc.scalar.activation(es[:, c0:], sp[:, c0:], AF.Exp, scale=SCALE)
                            if kd >= 0:
                                nc.vector.tensor_tensor(
                                    es[:, c0:c0 + 128], es[:, c0:c0 + 128],
                                    tri_sb[:], OP.mult)
                            za = zacc[jt % 2]
                            if jt < 2:
                                # first tile of this parity: copy (+ zero the
                                # never-written columns below c0)
                                if c0 > 0:
                                    nc.vector.memset(za[:, 0:c0], 0.0)
                                nc.vector.tensor_copy(za[:, c0:], es[:, c0:])
                            else:
                                nc.vector.tensor_tensor(
                                    za[:, c0:], za[:, c0:], es[:, c0:], OP.add)
                            vsl = v_sb[:, jt * 512 + hl * 128: jt * 512 + (hl + 1) * 128]
                            if c0 < 512:
                                nc.tensor.matmul(
                                    oT[:, c0:512], vsl, es[:, c0:512],
                                    start=(jt == 0), stop=(jt == njt - 1),
                                    skip_group_check=True)
                            c0r = max(c0, 512)
                            nc.tensor.matmul(
                                oT[:, c0r:], vsl, es[:, c0r:],
                                start=(jt == 0), stop=(jt == njt - 1),
                                skip_group_check=True)
                            ess[jt] = es

                        # software pipeline: score(jt+1) issues before pv(jt)
                        emit_score(0)
                        for jt in range(njt):
                            if jt + 1 < njt:
                                emit_score(jt + 1)
                            emit_exp_pv(jt)

                        zrow = psX.tile([1, 1024], f32, tag="mm", name=f"zr{b}{hl}")
                        for hf in range(2):
                            for par in range(2):
                                nc.tensor.matmul(
                                    zrow[:, hf * 512:(hf + 1) * 512], ones_sb[:],
                                    zacc[par][:, hf * 512:(hf + 1) * 512],
                                    start=(par == 0), stop=(par == 1))
                        zr = pSm.tile([1, 1024], f32, tag="zrec")
                        nc.vector.reciprocal(zr[:], zrow[:])
                        zbc = pZb.tile([128, 1024], f32, tag="zbc")
                        nc.gpsimd.partition_broadcast(zbc[:], zr[:])
                        nc.vector.tensor_tensor(otc[hl][:], oT[:], zbc[:], OP.mult)
                        if b == 0 and fillers:
                            fillers.pop(0)()
                            if fillers and hl >= 2:
                                fillers.pop(0)()
                    while b == 0 and fillers:
                        fillers.pop(0)()

                    # output projection for this q-block
                    for sg in range(4):  # groups of 2 row-tiles -> one DMA
                        ys = pYs.tile([128, 2 * D_MODEL], bf16, tag="ysb",
                                      name=f"ys{b}{sg}")
                        for si in range(2):
                            st = sg * 2 + si
                            for ncol in range(2):
                                yp = psX.tile([128, 1024], f32, tag="mm",
                                              name=f"y{b}{st}{ncol}")
                                for f in range(H_LOC):
                                    for hf in range(2):
                                        nc.tensor.matmul(
                                            yp[:, hf * 512:(hf + 1) * 512],
                                            otc[f][:, st * 128:(st + 1) * 128],
                                            wout_sb[:, f * D_MODEL + ncol * 1024 + hf * 512:
                                                    f * D_MODEL + ncol * 1024 + (hf + 1) * 512],
                                            start=(f == 0), stop=(f == H_LOC - 1))
                                dst = ys[:, si * D_MODEL + ncol * 1024:
                                         si * D_MODEL + (ncol + 1) * 1024]
                                if (st + ncol) % 2 == 0:
                                    nc.vector.tensor_copy(dst, yp[:])
                                else:
                                    nc.scalar.copy(dst, yp[:])
                        row0 = b * 1024 + sg * 256
                        nc.sync.dma_start(
                            out=y[row0:row0 + 256, :].rearrange(
                                "(si p) s -> p si s", p=128),
                            in_=ys.rearrange("p (si s) -> p si s", s=D_MODEL))

    nc.finalize()
    _BUILD_CACHE[key] = nc
    return nc


def _rope_mult():
    """r[s, d] = cos + sin rope multiplier, transposed to [64, SEQ]."""
    half = D_ROPE // 2
    theta = 1.0 / (ROPE_BASE ** (np.arange(0, D_HEAD, 2, dtype=np.float32) / D_HEAD))
    idx = np.arange(SEQ, dtype=np.float32)[:, None] * theta[None, :]
    r = np.tile(np.cos(idx[:, :half]), (1, 2)) + np.tile(np.sin(idx[:, :half]), (1, 2))
    return np.ascontiguousarray(r.T).astype(np.float32)  # [64, SEQ]


def make_inputs(x, W_dq, W_uq, W_dkv, W_uk, W_uv, W_qr, W_kr, g_q, g_kv, W_out, b_out):
    """Host-side sharding/packing: per-core input maps."""
    rT = _rope_mult()
    mult = np.empty((128, 2 * SEQ), np.float32)
    mult[0:64, 0:SEQ] = 1.0
    mult[64:128, 0:SEQ] = rT
    mult[0:64, SEQ:] = rT
    mult[64:128, SEQ:] = 1.0
    mult = mult.astype(BF16)

    # upper triangle: tri[p, c] = (c >= p); applied at the 128-wide diagonal
    # sub-block of every diagonal j-tile
    tri = (np.arange(128)[None, :] >= np.arange(128)[:, None]).astype(np.float32)
    tri = tri.astype(BF16)

    gq = g_q.astype(np.float32)[:, None]
    gkv = g_kv.astype(np.float32)[:, None]
    Wuq_g = W_uq * gq
    Wqr_g = W_qr * gq
    Wuk_g = W_uk * gkv
    Wuv_g = W_uv * gkv

    in_maps = []
    for core in range(N_CORES):
        b = core // 4
        g = core % 4
        heads = [4 * g + i for i in range(H_LOC)]

        xb = np.ascontiguousarray(x[b].T).astype(BF16)  # [d_model, seq]

        wq_pack = np.empty((D_KV, H_LOC * 128), np.float32)
        for hl, h in enumerate(heads):
            a = Wuq_g[:, h * 64:(h + 1) * 64]
            r = Wqr_g[:, h * 64:(h + 1) * 64]
            blk = np.concatenate([a, r], axis=1) if hl % 2 == 0 else np.concatenate([r, a], axis=1)
            wq_pack[:, hl * 128:(hl + 1) * 128] = blk

        wuk2 = np.empty((D_KV, 256), np.float32)
        wkr2 = np.empty((D_MODEL, 256), np.float32)
        for p in range(2):
            h0, h1 = heads[2 * p], heads[2 * p + 1]
            wuk2[:, p * 128: p * 128 + 64] = Wuk_g[:, h0 * 64:(h0 + 1) * 64]
            wuk2[:, p * 128 + 64: p * 128 + 128] = Wuk_g[:, h1 * 64:(h1 + 1) * 64]
            # rot halves swapped: odd head's rope block first
            wkr2[:, p * 128: p * 128 + 64] = W_kr[:, h1 * 64:(h1 + 1) * 64]
            wkr2[:, p * 128 + 64: p * 128 + 128] = W_kr[:, h0 * 64:(h0 + 1) * 64]

        wuv_pack = np.concatenate(
            [Wuv_g[:, h * 128:(h + 1) * 128] for h in heads], axis=1)
        wout_pack = np.concatenate(
            [W_out[h * 128:(h + 1) * 128, :] for h in heads], axis=0)

        in_maps.append({
            "xT": xb,
            "xq": np.ascontiguousarray(xb[:, g * 512:(g + 1) * 512]),
            "wdq": W_dq.astype(BF16),
            "wdkv": W_dkv.astype(BF16),
            "wq": wq_pack.astype(BF16),
            "wuk2": wuk2.astype(BF16),
            "wkr2": wkr2.astype(BF16),
            "wuv": wuv_pack.astype(BF16),
            "wout": wout_pack.astype(BF16),
            "mult": mult,
            "tri": tri,
        })
    return in_maps


def kernel(**inputs):
    inputs = {k: np.asarray(v) for k, v in inputs.items()}
    in_maps = make_inputs(
        inputs["x"], inputs["W_dq"], inputs["W_uq"], inputs["W_dkv"],
        inputs["W_uk"], inputs["W_uv"], inputs["W_qr"], inputs["W_kr"],
        inputs["g_q"], inputs["g_kv"], inputs["W_out"], inputs["b_out"])

    nc = build_program(reps=1)
    from concourse.bass_utils import run_bass_kernel_spmd
    res = run_bass_kernel_spmd(nc, in_maps, list(range(N_CORES)))

    b_out = inputs["b_out"].astype(np.float32)
    out = np.zeros((BATCH, SEQ, D_MODEL), np.float32)
    for core in range(N_CORES):
        out[core // 4] += res.results[core]["y"].astype(np.float32)
    out += b_out[None, None, :]
    return out


# revision 6
# speedup vs baseline: 11.0103x; 11.0103x over previous
"""MLA (multi-head latent attention) Trainium2 Bass kernel, v2.

Sharding: 8 cores = batch(2) x head-groups(4 heads each).
Latent projections computed on seq-quarters within a batch group and
all-gathered; heads tensor-parallel; final projection row-split with
host-side partial sum.

v2 vs v1:
- Attention PV uses V as the stationary operand with PSUM accumulation
  over j-tiles -> 96 wide matmuls instead of 544 narrow ones, and the
  output lands pre-transposed [d, q] so the 64 PE transposes are gone.
- Softmax denominator accumulated on DVE (f32) + one ones-matmul.
- 1024-wide moving operands for scores/PV/out-projection/kr halve the
  PE instruction count (sequencer was the bottleneck).
- Single [128,128] triangle applies the causal mask at every diagonal
  tile; sub-diagonal columns are never computed.
- Phase-B weights DMA'd at program start on the scalar queue so the
  collective stall doesn't serialize them.
"""

import sys
import numpy as np
import ml_dtypes

for _p in ("/opt/trn_rl_repo", "/root/.axon_site/_ro/trn_rl_repo"):
    if _p not in sys.path:
        sys.path.append(_p)

BF16 = ml_dtypes.bfloat16

D_MODEL = 2048
SEQ = 2048
BATCH = 2
N_HEADS = 16
D_HEAD = 128
D_KV = 512
D_ROPE = 64
ROPE_BASE = 10000.0
EPS = 1e-5
H_LOC = 4          # heads per core
N_CORES = 8

_BUILD_CACHE = {}


def build_program(reps: int = 1, split_coll: bool = True):
    key = (reps, split_coll)
    if key in _BUILD_CACHE:
        return _BUILD_CACHE[key]

    import concourse.bass as bass  # noqa: F401
    import concourse.mybir as mybir
    from concourse import bacc
    from concourse.tile import TileContext
    from contextlib import ExitStack

    f32 = mybir.dt.float32
    bf16 = mybir.dt.bfloat16
    AF = mybir.ActivationFunctionType
    OP = mybir.AluOpType

    nc = bacc.Bacc(num_devices=8)

    xT = nc.declare_dram_parameter("xT", [D_MODEL, SEQ], bf16, isOutput=False)
    wdq = nc.declare_dram_parameter("wdq", [D_MODEL, D_KV], bf16, isOutput=False)
    wdkv = nc.declare_dram_parameter("wdkv", [D_MODEL, D_KV], bf16, isOutput=False)
    wq = nc.declare_dram_parameter("wq", [D_KV, H_LOC * 128], bf16, isOutput=False)
    wuk2 = nc.declare_dram_parameter("wuk2", [D_KV, 2 * 128], bf16, isOutput=False)
    wkr2 = nc.declare_dram_parameter("wkr2", [D_MODEL, 2 * 128], bf16, isOutput=False)
    wuv = nc.declare_dram_parameter("wuv", [D_KV, H_LOC * 128], bf16, isOutput=False)
    wout = nc.declare_dram_parameter("wout", [H_LOC * 128, D_MODEL], bf16, isOutput=False)
    xq = nc.declare_dram_parameter("xq", [D_MODEL, 512], bf16, isOutput=False)
    mult = nc.declare_dram_parameter("mult", [128, 2 * SEQ], bf16, isOutput=False)
    tri = nc.declare_dram_parameter("tri", [128, 128], bf16, isOutput=False)
    y = nc.declare_dram_parameter("y", [SEQ, D_MODEL], bf16, isOutput=True)
    if split_coll:
        csrc_kv = nc.dram_tensor("csrc_kv", [D_KV, 512], bf16)
        cdst_kv = nc.dram_tensor("cdst_kv", [4, D_KV, 512], bf16)
        csrc_q = nc.dram_tensor("csrc_q", [D_KV, 512], bf16)
        cdst_q = nc.dram_tensor("cdst_q", [4, D_KV, 512], bf16)
    else:
        csrc = nc.dram_tensor("csrc", [2 * D_KV, 512], bf16)
        cdst = nc.dram_tensor("cdst", [4, 2 * D_KV, 512], bf16)

    SCALE = 1.0 / float(np.sqrt(np.float32(D_HEAD)))
    NKT = D_MODEL // 128    # 16 k-tiles over d_model
    NLT = D_KV // 128       # 4  k-tiles over latent
    NSN = SEQ // 512        # 4  sequence chunks (gather granularity)
    NB = SEQ // 1024        # 2  attention q-blocks

    with TileContext(nc) as tc, ExitStack() as top:
        pp = top.enter_context(tc.tile_pool(name="persist", bufs=1))
        qt_sb = pp.tile([128, H_LOC * SEQ], bf16, tag="qt")
        kt_sb = pp.tile([128, H_LOC * SEQ], bf16, tag="kt")
        v_sb = pp.tile([128, (SEQ // 128) * H_LOC * 128], bf16, tag="v")
        tri_sb = pp.tile([128, 128], bf16, tag="tri")
        ones_sb = pp.tile([128, 1], bf16, tag="ones")

        for _rep in range(reps):
            # early loads on the scalar queue (off the critical path)
            nc.scalar.dma_start(out=tri_sb[:], in_=tri[:, :])
            nc.gpsimd.memset(ones_sb[:], 1.0)

            # ---------------- Phase A: latents + projections ----------------
            with ExitStack() as pa:
                pA = pa.enter_context(tc.tile_pool(name="phA", bufs=1))
                psX = pa.enter_context(tc.tile_pool(name="psX", bufs=2, space="PSUM"))
                pakr = pa.enter_context(ExitStack())
                pAk = pakr.enter_context(tc.tile_pool(name="phAk", bufs=1))
                pXs = pakr.enter_context(tc.tile_pool(name="pXs", bufs=2))
                pa1 = pa.enter_context(ExitStack())
                pA1 = pa1.enter_context(tc.tile_pool(name="phA1", bufs=1))
                pCp = pa1.enter_context(tc.tile_pool(name="pCp", bufs=4))
                pCn = pa1.enter_context(tc.tile_pool(name="pCn", bufs=2))
                pSq = pa1.enter_context(tc.tile_pool(name="pSq", bufs=1))
                pSt = pa1.enter_context(tc.tile_pool(name="pSt", bufs=1))
                psS = pa1.enter_context(tc.tile_pool(name="psS", bufs=2, space="PSUM"))

                xq_sb = pA1.tile([128, NKT * 512], bf16, tag="xq")
                wdq_sb = pA1.tile([128, NKT * D_KV], bf16, tag="wdq")
                wdkv_sb = pA1.tile([128, NKT * D_KV], bf16, tag="wdkv")
                eps_sb = pA1.tile([1, 1], f32, tag="eps")
                cq_full = pA.tile([128, NLT * SEQ], bf16, tag="cqf")
                ckv_full = pA.tile([128, NLT * SEQ], bf16, tag="ckvf")
                wq_sb = pA.tile([128, NLT * 512], bf16, tag="wq")
                wuk2_sb = pA.tile([128, NLT * 256], bf16, tag="wuk2")
                wkr2_sb = pAk.tile([128, NKT * 256], bf16, tag="wkr2")
                wuv_sb = pA.tile([128, NLT * 512], bf16, tag="wuv")
                mult_sb = pA.tile([128, 2 * SEQ], bf16, tag="mult")

                def merged(dram, cols):
                    return dram.rearrange("(kt p) s -> p kt s", p=128)

                xq_v = xq_sb.rearrange("p (kt s) -> p kt s", s=512)
                for kt in range(4):
                    nc.sync.dma_start(
                        out=xq_v[:, kt:kt + 1],
                        in_=xq[kt * 128:(kt + 1) * 128, :].rearrange(
                            "(kt p) s -> p kt s", p=128))
                for kq in range(1, 4):
                    nc.sync.dma_start(
                        out=xq_v[:, kq * 4:(kq + 1) * 4],
                        in_=xq[kq * 512:(kq + 1) * 512, :].rearrange(
                            "(kt p) s -> p kt s", p=128))
                for lt in range(NLT):
                    nc.scalar.dma_start(
                        out=wdkv_sb.rearrange("p (kt s) -> p kt s", s=D_KV)[:, :, lt * 128:(lt + 1) * 128],
                        in_=wdkv[:, lt * 128:(lt + 1) * 128].rearrange(
                            "(kt p) s -> p kt s", p=128))
                for lt in range(NLT):
                    nc.scalar.dma_start(
                        out=wdq_sb.rearrange("p (kt s) -> p kt s", s=D_KV)[:, :, lt * 128:(lt + 1) * 128],
                        in_=wdq[:, lt * 128:(lt + 1) * 128].rearrange(
                            "(kt p) s -> p kt s", p=128))
                nc.scalar.dma_start(
                    out=wkr2_sb.rearrange("p (kt s) -> p kt s", s=256),
                    in_=merged(wkr2, 256))
                nc.scalar.dma_start(
                    out=wq_sb.rearrange("p (lt s) -> p lt s", s=512),
                    in_=wq.rearrange("(lt p) s -> p lt s", p=128))
                nc.scalar.dma_start(
                    out=wuk2_sb.rearrange("p (lt s) -> p lt s", s=256),
                    in_=wuk2.rearrange("(lt p) s -> p lt s", p=128))
                nc.scalar.dma_start(
                    out=wuv_sb.rearrange("p (lt s) -> p lt s", s=512),
                    in_=wuv.rearrange("(lt p) s -> p lt s", p=128))
                nc.scalar.dma_start(out=mult_sb[:], in_=mult[:, :])
                nc.gpsimd.memset(eps_sb[:], EPS)

                # --- local latent quarter from xq, rmsnorm, kv first ---
                for ci, (cname, wd_sb) in enumerate((("kv", wdkv_sb), ("q", wdq_sb))):
                    cps_l = []
                    sq = pSq.tile([128, NLT * 512], bf16, tag="sq", name=f"sq{cname}")
                    for lt in range(NLT):
                        cp = psX.tile([128, 512], f32, tag="mm")
                        for kt in range(NKT):
                            nc.tensor.matmul(
                                cp[:],
                                wd_sb[:, kt * D_KV + lt * 128: kt * D_KV + (lt + 1) * 128],
                                xq_sb[:, kt * 512:(kt + 1) * 512],
                                start=(kt == 0), stop=(kt == NKT - 1))
                        cps = pCp.tile([128, 512], f32, tag="cpre", name=f"cpre{cname}{lt}")
                        nc.vector.tensor_copy(cps[:], cp[:])
                        nc.vector.tensor_tensor(sq[:, lt * 512:(lt + 1) * 512],
                                                cps[:], cps[:], OP.mult)
                        cps_l.append(cps)
                    ss = psS.tile([1, 512], f32, tag="stat")
                    for lt in range(NLT):
                        nc.tensor.matmul(ss[:], ones_sb[:],
                                         sq[:, lt * 512:(lt + 1) * 512],
                                         start=(lt == 0), stop=(lt == NLT - 1))
                    lnv = pSt.tile([1, 512], f32, tag="ln")
                    nc.scalar.activation(lnv[:], ss[:], AF.Ln, scale=1.0 / D_KV, bias=eps_sb[:])
                    rstd = pSt.tile([1, 512], f32, tag="rstd")
                    nc.scalar.activation(rstd[:], lnv[:], AF.Exp, scale=-0.5)
                    bstd = pSt.tile([128, 512], f32, tag="bstd")
                    nc.gpsimd.partition_broadcast(bstd[:], rstd[:])
                    cn = pCn.tile([128, NLT * 512], bf16, tag="cnloc", name=f"cnloc{cname}")
                    for lt in range(NLT):
                        nc.vector.tensor_tensor(cn[:, lt * 512:(lt + 1) * 512],
                                                cps_l[lt][:], bstd[:], OP.mult)
                    cn_v = cn.rearrange("p (lt s) -> p lt s", s=512)
                    if split_coll:
                        cs = csrc_kv if cname == "kv" else csrc_q
                        nc.sync.dma_start(
                            out=cs.rearrange("(lt p) s -> p lt s", p=128), in_=cn_v)
                    else:
                        nc.sync.dma_start(
                            out=csrc[ci * D_KV:(ci + 1) * D_KV, :].rearrange(
                                "(lt p) s -> p lt s", p=128), in_=cn_v)
                    if split_coll and cname == "kv":
                        nc.gpsimd.collective_compute(
                            "AllGather", OP.bypass,
                            replica_groups=[[0, 1, 2, 3], [4, 5, 6, 7]],
                            ins=[csrc_kv[:, :]], outs=[cdst_kv[:, :, :]])

                if split_coll:
                    nc.gpsimd.collective_compute(
                        "AllGather", OP.bypass,
                        replica_groups=[[0, 1, 2, 3], [4, 5, 6, 7]],
                        ins=[csrc_q[:, :]], outs=[cdst_q[:, :, :]])
                else:
                    nc.gpsimd.collective_compute(
                        "AllGather", OP.bypass,
                        replica_groups=[[0, 1, 2, 3], [4, 5, 6, 7]],
                        ins=[csrc[:, :]], outs=[cdst[:, :, :]])

                pa1.close()

                # --- kr projections from streamed xT (fills collective window) ---
                for half in range(2):
                    kps = [psX.tile([128, 1024], f32, tag="mm", name=f"kr{half}{p}")
                           for p in range(2)]
                    for kq in range(4):
                        xtile = pXs.tile([128, 4 * 1024], bf16, tag="xs", name=f"xs{half}{kq}")
                        nc.sync.dma_start(
                            out=xtile.rearrange("p (kt s) -> p kt s", s=1024),
                            in_=xT[kq * 512:(kq + 1) * 512,
                                   half * 1024:(half + 1) * 1024].rearrange(
                                       "(kt p) s -> p kt s", p=128))
                        for ki in range(4):
                            kt = kq * 4 + ki
                            for p in range(2):
                                for hf in range(2):
                                    nc.tensor.matmul(
                                        kps[p][:, hf * 512:(hf + 1) * 512],
                                        wkr2_sb[:, kt * 256 + p * 128: kt * 256 + (p + 1) * 128],
                                        xtile[:, ki * 1024 + hf * 512: ki * 1024 + (hf + 1) * 512],
                                        start=(kt == 0), stop=(kt == NKT - 1))
                    for p in range(2):
                        h0, h1 = 2 * p, 2 * p + 1
                        kp = kps[p]
                        a, z = half * 1024, (half + 1) * 1024
                        m0 = mult_sb[:, 0 * SEQ + a: 0 * SEQ + z]
                        m1 = mult_sb[:, 1 * SEQ + a: 1 * SEQ + z]
                        k0 = kt_sb[:, h0 * SEQ + a: h0 * SEQ + z]
                        k1 = kt_sb[:, h1 * SEQ + a: h1 * SEQ + z]
                        nc.vector.tensor_tensor(k0[64:128, :], kp[64:128, :], m0[64:128, :], OP.mult)
                        nc.vector.tensor_tensor(k1[0:64, :], kp[0:64, :], m1[0:64, :], OP.mult)

                pakr.close()
                psY = pa.enter_context(tc.tile_pool(name="psY", bufs=2, space="PSUM"))

                # --- unpack gathered latents (kv first, sn-major) ---
                for ci, cfull in ((0, ckv_full), (1, cq_full)):
                    cf_v = cfull.rearrange("p (lt sn s) -> p lt sn s", sn=NSN, s=512)
                    for sn in range(NSN):
                        if split_coll:
                            cd = cdst_kv if ci == 0 else cdst_q
                            in_v = cd[sn].rearrange("(lt p) s -> p lt s", p=128)
                        else:
                            in_v = cdst[sn, ci * D_KV:(ci + 1) * D_KV, :].rearrange(
                                "(lt p) s -> p lt s", p=128)
                        nc.sync.dma_start(out=cf_v[:, :, sn], in_=in_v)

                # --- K/V/Q projections, emitted per seq-half so block-0
                # attention can start while the second half projects ---
                def emit_kchain(pr, p):
                    a, z = pr * 1024, (pr + 1) * 1024
                    if True:
                        h0, h1 = 2 * p, 2 * p + 1
                        up = psX.tile([128, 1024], f32, tag="mm", name=f"uk{pr}{p}")
                        for lt in range(NLT):
                            for hf in range(2):
                                nc.tensor.matmul(
                                    up[:, hf * 512:(hf + 1) * 512],
                                    wuk2_sb[:, lt * 256 + p * 128: lt * 256 + (p + 1) * 128],
                                    ckv_full[:, lt * SEQ + a + hf * 512:
                                             lt * SEQ + a + (hf + 1) * 512],
                                    start=(lt == 0), stop=(lt == NLT - 1))
                        m0 = mult_sb[:, 0 * SEQ + a: 0 * SEQ + z]
                        m1 = mult_sb[:, 1 * SEQ + a: 1 * SEQ + z]
                        k0 = kt_sb[:, h0 * SEQ + a: h0 * SEQ + z]
                        k1 = kt_sb[:, h1 * SEQ + a: h1 * SEQ + z]
                        nc.vector.tensor_tensor(k0[0:64, :], up[0:64, :], m0[0:64, :], OP.mult)
                        nc.vector.tensor_tensor(k1[64:128, :], up[64:128, :], m1[64:128, :], OP.mult)

                def emit_vproj(sn):
                    for st in range(4):
                        s_tile = sn * 4 + st
                        vp = psY.tile([128, 512], f32, tag="vo")
                        for lt in range(NLT):
                            nc.tensor.matmul(
                                vp[:],
                                ckv_full[:, lt * SEQ + sn * 512 + st * 128:
                                         lt * SEQ + sn * 512 + (st + 1) * 128],
                                wuv_sb[:, lt * 512:(lt + 1) * 512],
                                start=(lt == 0), stop=(lt == NLT - 1))
                        nc.vector.tensor_copy(
                            v_sb[:, s_tile * 512:(s_tile + 1) * 512], vp[:])

                def emit_qchain(pr, hl):
                    a, z = pr * 1024, (pr + 1) * 1024
                    if True:
                        qp = psX.tile([128, 1024], f32, tag="mm", name=f"q{pr}{hl}")
                        for lt in range(NLT):
                            for hf in range(2):
                                nc.tensor.matmul(
                                    qp[:, hf * 512:(hf + 1) * 512],
                                    wq_sb[:, lt * 512 + hl * 128: lt * 512 + (hl + 1) * 128],
                                    cq_full[:, lt * SEQ + a + hf * 512:
                                            lt * SEQ + a + (hf + 1) * 512],
                                    start=(lt == 0), stop=(lt == NLT - 1))
                        nc.vector.tensor_tensor(
                            qt_sb[:, hl * SEQ + a: hl * SEQ + z],
                            qp[:], mult_sb[:, (hl % 2) * SEQ + a: (hl % 2) * SEQ + z],
                            OP.mult)

                emit_kchain(0, 0)
                emit_kchain(0, 1)
                for sn in range(NSN):
                    emit_vproj(sn)
                for hl in range(H_LOC):
                    emit_qchain(0, hl)
                # second-half K/Q chains are woven between block-0 attention
                # heads below
                fillers = [lambda p=p: emit_kchain(1, p) for p in range(2)]
                fillers += [lambda hl=hl: emit_qchain(1, hl) for hl in range(H_LOC)]

            # ------------- Phase B: attention + output projection -------------
            # (same pool scope as phase A so attention interleaves with the
            # trailing projections; psX/psY carry the PSUM traffic)
            if True:
                pb = pa.enter_context(ExitStack())
                pB = pb.enter_context(tc.tile_pool(name="pB", bufs=1))
                pEs = pb.enter_context(tc.tile_pool(name="pEs", bufs=8))
                pZa = pb.enter_context(tc.tile_pool(name="pZa", bufs=2))
                pZb = pb.enter_context(tc.tile_pool(name="pZb", bufs=2))
                pOt = pb.enter_context(tc.tile_pool(name="pOt", bufs=2))
                pYs = pb.enter_context(tc.tile_pool(name="pYs", bufs=2))
                pSm = pb.enter_context(tc.tile_pool(name="pSm", bufs=2))

                wout_sb = pB.tile([128, NLT * D_MODEL], bf16, tag="wout")
                nc.scalar.dma_start(
                    out=wout_sb.rearrange("p (f s) -> p f s", s=D_MODEL),
                    in_=wout.rearrange("(f p) s -> p f s", p=128))

                for b in range(NB):
                    njt = (b + 1) * 8
                    otc = [pOt.tile([128, 1024], bf16, tag=f"otc{f}", name=f"otc{b}_{f}")
                           for f in range(H_LOC)]
                    for hl in range(H_LOC):
                        oT = psY.tile([128, 1024], f32, tag="vo", name=f"oT{b}{hl}")
                        zacc = [pZa.tile([128, 1024], bf16, tag=f"zacc{par}",
                                         name=f"za{b}{hl}{par}") for par in range(2)]
                        sps, ess = {}, {}

                        def emit_score(jt):
                            kd = jt - b * 8
                            c0 = max(kd, 0) * 128
                            sp = psX.tile([128, 1024], f32, tag="mm", name=f"s{b}{hl}{jt}")
                            ksl = kt_sb[:, hl * SEQ + jt * 128: hl * SEQ + (jt + 1) * 128]
                            if c0 < 512:
                                nc.tensor.matmul(
                                    sp[:, c0:512],
                                    ksl,
                                    qt_sb[:, hl * SEQ + b * 1024 + c0:
                                          hl * SEQ + b * 1024 + 512],
                                    start=True, stop=True)
                            c0r = max(c0, 512)
                            nc.tensor.matmul(
                                sp[:, c0r:],
                                ksl,
                                qt_sb[:, hl * SEQ + b * 1024 + c0r: hl * SEQ + (b + 1) * 1024],
                                start=True, stop=True)
                            sps[jt] = sp

                        def emit_exp_pv(jt):
                            kd = jt - b * 8
                            c0 = max(kd, 0) * 128
                            sp = sps.pop(jt)
                            es = pEs.tile([128, 1024], bf16, tag="es", name=f"e{b}{hl}{jt}")
                            nc.scalar.activation(es[:, c0:], sp[:, c0:], AF.Exp, scale=SCALE)
                            if kd >= 0:
                                nc.vector.tensor_tensor(
                                    es[:, c0:c0 + 128], es[:, c0:c0 + 128],
                                    tri_sb[:], OP.mult)
                            za = zacc[jt % 2]
                            if jt < 2:
                                # first tile of this parity: copy (+ zero the
                                # never-written columns below c0)
                                if c0 > 0:
                                    nc.vector.memset(za[:, 0:c0], 0.0)
                                nc.vector.tensor_copy(za[:, c0:], es[:, c0:])
                            else:
                                nc.vector.tensor_tensor(
                                    za[:, c0:], za[:, c0:], es[:, c0:], OP.add)
                            vsl = v_sb[:, jt * 512 + hl * 128: jt * 512 + (hl + 1) * 128]
                            if c0 < 512:
                                nc.tensor.matmul(
                                    oT[:, c0:512], vsl, es[:, c0:512],
                                    start=(jt == 0), stop=(jt == njt - 1),
                                    skip_group_check=True)
                            c0r = max(c0, 512)
                            nc.tensor.matmul(
                                oT[:, c0r:], vsl, es[:, c0r:],
                                start=(jt == 0), stop=(jt == njt - 1),
                                skip_group_check=True)
                            ess[jt] = es

                        # software pipeline: score(jt+1) issues before pv(jt)
                        emit_score(0)
                        for jt in range(njt):
                            if jt + 1 < njt:
                                emit_score(jt + 1)
                            emit_exp_pv(jt)

                        zrow = psX.tile([1, 1024], f32, tag="mm", name=f"zr{b}{hl}")
                        for hf in range(2):
                            for par in range(2):
                                nc.tensor.matmul(
                                    zrow[:, hf * 512:(hf + 1) * 512], ones_sb[:],
                                    zacc[par][:, hf * 512:(hf + 1) * 512],
                                    start=(par == 0), stop=(par == 1))
                        zr = pSm.tile([1, 1024], f32, tag="zrec")
                        nc.vector.reciprocal(zr[:], zrow[:])
                        zbc = pZb.tile([128, 1024], f32, tag="zbc")
                        nc.gpsimd.partition_broadcast(zbc[:], zr[:])
                        nc.vector.tensor_tensor(otc[hl][:], oT[:], zbc[:], OP.mult)
                        if b == 0 and fillers:
                            fillers.pop(0)()
                            if fillers and hl >= 2:
                                fillers.pop(0)()
                    while b == 0 and fillers:
                        fillers.pop(0)()

                    # output projection for this q-block
                    for sg in range(8):  # one row-tile per DMA (short tail)
                        ys = pYs.tile([128, D_MODEL], bf16, tag="ysb",
                                      name=f"ys{b}{sg}")
                        for si in range(1):
                            st = sg
                            for ncol in range(2):
                                yp = psX.tile([128, 1024], f32, tag="mm",
                                              name=f"y{b}{st}{ncol}")
                                for f in range(H_LOC):
                                    for hf in range(2):
                                        nc.tensor.matmul(
                                            yp[:, hf * 512:(hf + 1) * 512],
                                            otc[f][:, st * 128:(st + 1) * 128],
                                            wout_sb[:, f * D_MODEL + ncol * 1024 + hf * 512:
                                                    f * D_MODEL + ncol * 1024 + (hf + 1) * 512],
                                            start=(f == 0), stop=(f == H_LOC - 1))
                                dst = ys[:, si * D_MODEL + ncol * 1024:
                                         si * D_MODEL + (ncol + 1) * 1024]
                                if (st + ncol) % 2 == 0:
                                    nc.vector.tensor_copy(dst, yp[:])
                                else:
                                    nc.scalar.copy(dst, yp[:])
                        row0 = b * 1024 + sg * 128
                        nc.sync.dma_start(out=y[row0:row0 + 128, :], in_=ys[:])

    nc.finalize()
    _BUILD_CACHE[key] = nc
    return nc


def _rope_mult():
    """r[s, d] = cos + sin rope multiplier, transposed to [64, SEQ]."""
    half = D_ROPE // 2
    theta = 1.0 / (ROPE_BASE ** (np.arange(0, D_HEAD, 2, dtype=np.float32) / D_HEAD))
    idx = np.arange(SEQ, dtype=np.float32)[:, None] * theta[None, :]
    r = np.tile(np.cos(idx[:, :half]), (1, 2)) + np.tile(np.sin(idx[:, :half]), (1, 2))
    return np.ascontiguousarray(r.T).astype(np.float32)  # [64, SEQ]


def make_inputs(x, W_dq, W_uq, W_dkv, W_uk, W_uv, W_qr, W_kr, g_q, g_kv, W_out, b_out):
    """Host-side sharding/packing: per-core input maps."""
    rT = _rope_mult()
    mult = np.empty((128, 2 * SEQ), np.float32)
    mult[0:64, 0:SEQ] = 1.0
    mult[64:128, 0:SEQ] = rT
    mult[0:64, SEQ:] = rT
    mult[64:128, SEQ:] = 1.0
    mult = mult.astype(BF16)

    # upper triangle: tri[p, c] = (c >= p); applied at the 128-wide diagonal
    # sub-block of every diagonal j-tile
    tri = (np.arange(128)[None, :] >= np.arange(128)[:, None]).astype(np.float32)
    tri = tri.astype(BF16)

    gq = g_q.astype(np.float32)[:, None]
    gkv = g_kv.astype(np.float32)[:, None]
    Wuq_g = W_uq * gq
    Wqr_g = W_qr * gq
    Wuk_g = W_uk * gkv
    Wuv_g = W_uv * gkv

    in_maps = []
    for core in range(N_CORES):
        b = core // 4
        g = core % 4
        heads = [4 * g + i for i in range(H_LOC)]

        xb = np.ascontiguousarray(x[b].T).astype(BF16)  # [d_model, seq]

        wq_pack = np.empty((D_KV, H_LOC * 128), np.float32)
        for hl, h in enumerate(heads):
            a = Wuq_g[:, h * 64:(h + 1) * 64]
            r = Wqr_g[:, h * 64:(h + 1) * 64]
            blk = np.concatenate([a, r], axis=1) if hl % 2 == 0 else np.concatenate([r, a], axis=1)
            wq_pack[:, hl * 128:(hl + 1) * 128] = blk

        wuk2 = np.empty((D_KV, 256), np.float32)
        wkr2 = np.empty((D_MODEL, 256), np.float32)
        for p in range(2):
            h0, h1 = heads[2 * p], heads[2 * p + 1]
            wuk2[:, p * 128: p * 128 + 64] = Wuk_g[:, h0 * 64:(h0 + 1) * 64]
            wuk2[:, p * 128 + 64: p * 128 + 128] = Wuk_g[:, h1 * 64:(h1 + 1) * 64]
            # rot halves swapped: odd head's rope block first
            wkr2[:, p * 128: p * 128 + 64] = W_kr[:, h1 * 64:(h1 + 1) * 64]
            wkr2[:, p * 128 + 64: p * 128 + 128] = W_kr[:, h0 * 64:(h0 + 1) * 64]

        wuv_pack = np.concatenate(
            [Wuv_g[:, h * 128:(h + 1) * 128] for h in heads], axis=1)
        wout_pack = np.concatenate(
            [W_out[h * 128:(h + 1) * 128, :] for h in heads], axis=0)

        in_maps.append({
            "xT": xb,
            "xq": np.ascontiguousarray(xb[:, g * 512:(g + 1) * 512]),
            "wdq": W_dq.astype(BF16),
            "wdkv": W_dkv.astype(BF16),
            "wq": wq_pack.astype(BF16),
            "wuk2": wuk2.astype(BF16),
            "wkr2": wkr2.astype(BF16),
            "wuv": wuv_pack.astype(BF16),
            "wout": wout_pack.astype(BF16),
            "mult": mult,
            "tri": tri,
        })
    return in_maps


def kernel(**inputs):
    inputs = {k: np.asarray(v) for k, v in inputs.items()}
    in_maps = make_inputs(
        inputs["x"], inputs["W_dq"], inputs["W_uq"], inputs["W_dkv"],
        inputs["W_uk"], inputs["W_uv"], inputs["W_qr"], inputs["W_kr"],
        inputs["g_q"], inputs["g_kv"], inputs["W_out"], inputs["b_out"])

    nc = build_program(reps=1)
    from concourse.bass_utils import run_bass_kernel_spmd
    res = run_bass_kernel_spmd(nc, in_maps, list(range(N_CORES)))

    b_out = inputs["b_out"].astype(np.float32)
    out = np.zeros((BATCH, SEQ, D_MODEL), np.float32)
    for core in range(N_CORES):
        out[core // 4] += res.results[core]["y"].astype(np.float32)
    out += b_out[None, None, :]
    return out
